# revision 53
# baseline (speedup 1.0000x reference)
"""Multi-head attention (axis-swapped variant) on 8 Trainium2 NeuronCores.

Reference semantics (EMB=1024): 64 effective heads of size 16 acting on the
d_head axis, causal softmax scaled by 1/sqrt(16), projections Wq/Wk/Wv,
output projection Wo + bo.

Sharding: core c = 4*b + g handles batch b and head-group g (16 heads =
256 contiguous projection columns). Each core returns a partial output
[1024, 1024]; the host sums the 4 group partials per batch and adds bo.

Per-core pipeline:
- bf16 Q/K/V projections (fp32 PSUM accumulate); the first query-half Q/K
  wave runs first so the fp8 reshape roundtrip starts early
- Q/K evacuated to fp8e4m3 with columns pre-ordered (e,h); a DRAM
  roundtrip reshapes them to [8, qk, 2, 16, 512]-per-half so score
  matmuls run in fp8 DoubleRow mode (contraction 16 = 8 part x 2 k-tiles)
- causal diag masking post-exp on GPSIMD (SBUF-only engine) via a
  lower-triangular constant multiply
- softmax exp split across three engines (greedy balance): ACT true exp,
  DVE/Pool use the Schraudolph bit-trick (int16(x*A+B) bitcast to bf16);
  score tiles are single-PSUM-bank [128, 2, 256]
- ctx accumulated transposed (out [128 queries, 16] per head) plus
  free-size-1 denominator matmuls against a ones column
- one merged attention loop over both halves; second-half Q/K projection,
  V projection tail, and the per-query-block epilogue (PE transpose,
  bf16 out-projection, store) run as fillers through one spare PSUM bank
"""

import numpy as np
import ml_dtypes

import concourse.bass as bass
import concourse.mybir as mybir
import concourse.tile as tile
from concourse.bass_utils import run_bass_kernel_spmd

F32 = mybir.dt.float32
BF16 = mybir.dt.bfloat16
F8 = mybir.dt.float8e4
I16 = mybir.dt.int16
BF = ml_dtypes.bfloat16
F8NP = ml_dtypes.float8_e4m3

EMB = 1024
SEQ = 1024
BATCH = 2
NG = 4            # head groups (cores per batch)
HPG = 16          # heads per group/core
DH = 16           # per-head feature size
GCOLS = HPG * DH  # 256 projection columns per core

DR = mybir.MatmulPerfMode.DoubleRow
MULT = mybir.AluOpType.mult
ADD = mybir.AluOpType.add
EXPF = mybir.ActivationFunctionType.Exp

# Schraudolph exp: bf16(bitcast_int16(s * A_S + B_S)) ~= exp(0.25 * s)
A_S = float(np.float32(0.25 * 128.0 / np.log(2.0)))
B_S = 16248.0

N_WARM = 8  # PE p-state warmup matmuls while input DMAs land
SW_BUFS = 4
AT_BUFS = 40


def split_excess_waits(nc, cap=1):
    """This container's walrus rejects instructions carrying more than a few
    semaphore waits (and bass's own model says one). Relocate excess waits
    onto preceding same-engine EventSemaphore instructions."""

    def fix_block(bb, dummy):
        insts = bb.instructions
        i = 0
        while i < len(insts):
            inst = insts[i]
            si = inst.sync_info
            waits = list(si.on_wait) if si is not None and si.on_wait else []
            if len(waits) > cap:
                eng = nc.engines[inst.engine]
                excess, keep = waits[:-cap], waits[-cap:]
                si.on_wait = keep
                pos = i
                for j in range(0, len(excess), cap):
                    chunk = excess[j : j + cap]
                    ev = eng.wait_ge(dummy, 1)
                    cur_list = nc.cur_bb.bb.instructions
                    assert cur_list[-1] is ev.ins
                    cur_list.pop()
                    ev.ins.sync_info.on_wait = chunk
                    insts.insert(pos, ev.ins)
                    pos += 1
                    i += 1
            i += 1

    with nc.semaphore("waitfix_dummy") as dummy:
        for f in nc.m.functions:
            for bb in f.blocks:
                fix_block(bb, dummy)


def _bcast(ap, dim, count):
    """Insert a stride-0 dim at position `dim` of an AP."""
    new_ap = list(ap.ap)
    new_ap.insert(dim, [0, count])
    return bass.AP(tensor=ap.tensor, offset=ap.offset, ap=new_ap)


class VecSplit:
    """Greedy load-balancing of exp/copy work across ACT / DVE / Pool."""

    def __init__(self, nc):
        self.nc = nc
        self.load = {"act": 0.0, "dve": 0.0, "pool": 0.0}
        self.exp_i = 0
        self.exp_cost = {
            "act": lambda r: r * 0.8333 + 185.0,
            "dve": lambda r: r * 1.0417 + 125.0,
        }

    def exp(self, at, ati, sw, rows):
        eng = ("act", "dve")[self.exp_i % 2]
        self.exp_i += 1
        self.load[eng] += self.exp_cost[eng](rows)
        if eng == "act":
            self.nc.scalar.activation(at, sw, EXPF, scale=0.25)
        else:
            self.nc.vector.tensor_scalar(ati, sw, A_S, B_S, MULT, ADD)

    def copy(self, out, in_, costs):
        eng = min(costs, key=lambda e: self.load[e] + costs[e])
        self.load[eng] += costs[eng]
        if eng == "act":
            self.nc.scalar.copy(out, in_)
        elif eng == "dve":
            self.nc.vector.tensor_copy(out, in_)
        else:
            self.nc.gpsimd.tensor_copy(out, in_)
        return eng

    def add(self, eng, ns):
        self.load[eng] += ns


def build_nc():
    nc = bass.Bass()
    xT_d = nc.declare_dram_parameter("xT", [EMB, SEQ], BF16, isOutput=False)
    wq_d = nc.declare_dram_parameter("wq", [EMB, GCOLS], BF16, isOutput=False)
    wk_d = nc.declare_dram_parameter("wk", [EMB, GCOLS], BF16, isOutput=False)
    wv_d = nc.declare_dram_parameter("wv", [EMB, GCOLS], BF16, isOutput=False)
    wo_d = nc.declare_dram_parameter("wo", [GCOLS, EMB], BF16, isOutput=False)
    id_d = nc.declare_dram_parameter("idm", [128, 256], BF16, isOutput=False)
    y_d = nc.declare_dram_parameter("y", [SEQ, EMB], BF16, isOutput=True)

    with tile.TileContext(nc) as tc:
        with (
            tc.tile_pool(name="big", bufs=1) as big,
            tc.tile_pool(name="att", bufs=AT_BUFS) as att,
            tc.tile_pool(name="work", bufs=4) as work,
            tc.tile_pool(name="dram", bufs=1, space="DRAM") as dram,
        ):
            # ---- input DMAs (order = SP queue order; no waits on any) ----
            xT_r = xT_d[:].rearrange("(kb p) m -> p kb m", p=128)
            XT = big.tile([128, 8, SEQ], BF16)
            WQ = big.tile([128, 8, GCOLS], BF16)
            WK = big.tile([128, 8, GCOLS], BF16)
            WV = big.tile([128, 8, GCOLS], BF16)
            WO = big.tile([128, 2, EMB], BF16)
            IDM = big.tile([128, 256], BF16)

            def xchunk(ci, mh):
                sl = (slice(None), slice(2 * ci, 2 * ci + 2),
                      slice(512 * mh, 512 * mh + 512))
                nc.sync.dma_start(XT[sl], xT_r[sl])

            nc.sync.dma_start(WQ[:], wq_d[:].rearrange("(kb p) n -> p kb n", p=128))
            xchunk(0, 0)
            nc.sync.dma_start(IDM[:], id_d[:])
            nc.sync.dma_start(WK[:], wk_d[:].rearrange("(kb p) n -> p kb n", p=128))
            xchunk(1, 0)
            xchunk(2, 0)
            xchunk(3, 0)
            nc.sync.dma_start(WV[:], wv_d[:].rearrange("(kb p) n -> p kb n", p=128))

            IDENT = IDM[:, 0:128]
            MASK = IDM[:, 128:256]   # lower-tri ones: key p <= query m

            QK8 = big.tile([128, 2, 2, SEQ], F8)       # (p=col, ct, qk, m)
            # per m-half fp8 score operands: (p8, qk, i, h, m)
            QKT8h = [
                big.tile([8, 2, 2, HPG, 512], F8, name=f"qkt8h{mh}")
                for mh in range(2)
            ]
            VA = big.tile([128, 8, HPG, DH], BF16)     # (p=key, kb, h, e)
            ONES = big.tile([128, 1], BF16)
            nc.gpsimd.memset(ONES[:], 1.0)
            ZL = big.tile([8, 2, 128], F8)
            nc.gpsimd.memset(ZL[:], 0.0)
            ZR = big.tile([8, 2, 512], F8)
            nc.gpsimd.memset(ZR[:], 0.0)
            CN = big.tile([128, 8, GCOLS], BF16)       # normalized ctx per qq
            qk8_d = dram.tile([2, 2, 2, 128, 512], F8)  # (mh, qk, ct, p, m)

            xs = VecSplit(nc)

            def zero_mm(out_ap):
                nc.tensor.matmul(out_ap, ZL[:], ZR[:], start=True, stop=False,
                                 perf_mode=DR, skip_group_check=True)

            def rt_dma(qki, mh):
                eng = nc.scalar if (qki == 0 and mh == 0) else nc.sync
                eng.dma_start(
                    qk8_d[mh, qki].rearrange("ct p m -> p ct m"),
                    QK8[:, :, qki, 512 * mh : 512 * mh + 512],
                )
                eng.dma_start(
                    QKT8h[mh][:, qki],
                    qk8_d[mh, qki].rearrange("i (p8 h) m -> p8 i h m", p8=8),
                )

            def proj_mms(pq, qki, ct, mh):
                Wt = (WQ, WK)[qki]
                for kb in range(8):
                    nc.tensor.matmul(
                        pq[:, 0:512],
                        Wt[:, kb, 128 * ct : 128 * ct + 128],
                        XT[:, kb, 512 * mh : 512 * mh + 512],
                        start=(kb == 0),
                        stop=(kb == 7),
                    )

            def proj_evac(pq, qki, ct, mh):
                xs.copy(
                    QK8[:, ct, qki, 512 * mh : 512 * mh + 512], pq[:, 0:512],
                    {"act": 612.0, "dve": 660.0},
                )
                if ct == 1:
                    rt_dma(qki, mh)

            def v_mms(pv, mt):
                for kb in range(8):
                    nc.tensor.matmul(
                        pv[:, 0:GCOLS],
                        XT[:, kb, 128 * mt : 128 * mt + 128],
                        WV[:, kb, :],
                        start=(kb == 0),
                        stop=(kb == 7),
                    )

            def v_evac(pv, mt):
                xs.copy(
                    VA[:, mt, :, :],
                    pv[:, 0:GCOLS].rearrange("p (h e) -> p h e", e=DH),
                    {"dve": 520.0, "act": 560.0},
                )

            # ---- P0: PE p-state warmup on zeros while DMAs land ----
            with tc.tile_pool(name="ps_w", bufs=1, space="PSUM") as ps_w:
                WARM = ps_w.tile([128, 512], F32, tag="warm")
                for _ in range(N_WARM):
                    nc.tensor.matmul(WARM[:], ZL[:], ZR[:], start=True, stop=True,
                                     perf_mode=DR, skip_group_check=True)

            # ---- P1: Q/K proj mh0 wave, then V mt0..3 ----
            with tc.tile_pool(name="ps_p", bufs=1, space="PSUM") as ps_p:
                pqs = {}
                for qki in range(2):
                    for ct in range(2):
                        pqs[(qki, ct)] = ps_p.tile(
                            [128, 512], F32, tag=f"pp{qki}{ct}",
                            name=f"pq{qki}{ct}",
                        )
                for kb in range(8):
                    for qki, Wt in enumerate((WQ, WK)):
                        for ct in range(2):
                            nc.tensor.matmul(
                                pqs[(qki, ct)][:],
                                Wt[:, kb, 128 * ct : 128 * ct + 128],
                                XT[:, kb, 0:512],
                                start=(kb == 0),
                                stop=(kb == 7),
                            )
                for qki in range(2):
                    for ct in range(2):
                        xs.copy(
                            QK8[:, ct, qki, 0:512], pqs[(qki, ct)][:],
                            {("act", "dve")[qki]: (612.0, 660.0)[qki]},
                        )
                    rt_dma(qki, 0)
                for ci in range(4):
                    xchunk(ci, 1)
                nc.sync.dma_start(
                    WO[:], wo_d[:].rearrange("(ch p) n -> p ch n", p=128))
                for mt in range(4):
                    pv = ps_p.tile([128, 512], F32, tag="pv", name=f"pv{mt}",
                                   bufs=2)
                    v_mms(pv, mt)
                    v_evac(pv, mt)

            # ---- P2: merged attention over both halves + fillers ----
            with tc.tile_pool(name="ps_a", bufs=1, space="PSUM") as pool:

                fill_items = [("p", 0, 0), ("p", 0, 1), ("p", 1, 0),
                              ("p", 1, 1), ("v", 4, 0), ("v", 5, 0),
                              ("v", 6, 0), ("v", 7, 0)]
                fill_state = {"i": 0, "pending": None}

                def fill_step():
                    if fill_state["pending"] is not None:
                        kind, a, b, tile_ = fill_state["pending"]
                        v_evac(tile_, a)
                        fill_state["pending"] = None
                    i = fill_state["i"]
                    if i < len(fill_items):
                        fill_state["i"] = i + 1
                        if fill_items[i] is None:
                            return
                        kind, a, b = fill_items[i]
                        t = pool.tile([128, 512], F32, tag="fill", bufs=1,
                                      name="fillt")
                        if kind == "p":
                            # early iterations, exp traffic still light:
                            # evac+rt immediately so QKT8h[1] lands in time
                            proj_mms(t, a, b, 1)
                            proj_evac(t, a, b, 1)
                        else:
                            v_mms(t, a)
                            fill_state["pending"] = (kind, a, b, t)

                steps = [(0, kb) for kb in range(4)] + \
                        [(1, kb) for kb in range(8)]

                half = {}

                def setup_half(ic):
                    DEN = pool.tile([128, 512], F32, tag="den", bufs=1,
                                    name=f"den{ic}")
                    zero_mm(DEN[:])
                    CTXT = []
                    for hf in range(2):
                        t = pool.tile([128, 2, GCOLS], F32, tag="ctx", bufs=2,
                                      name=f"ctx{ic}{hf}")
                        zero_mm(t[:])
                        CTXT.append(t)
                    half[ic] = (
                        DEN[:, 0:64].rearrange("p (qb h) -> p qb h", h=HPG),
                        CTXT,
                    )

                def normalize(ic, qb):
                    den_v, CTXT = half[ic]
                    qq = 4 * ic + qb
                    ctx3 = CTXT[qb // 2][:, qb % 2, :].rearrange(
                        "p (h e) -> p h e", e=DH)
                    RG = work.tile([128, HPG], F32, tag="rg", name="rg")
                    nc.vector.reciprocal(out=RG[:], in_=den_v[:, qb, :])
                    nc.vector.tensor_tensor(
                        CN[:, qq, :].rearrange("p (h e) -> p h e", e=DH),
                        ctx3,
                        _bcast(RG[:], 2, DH),
                        op=MULT,
                    )
                    xs.add("dve", 900)

                def emit_ctx_pr(ic, kb, ATs, pr):
                    den_v, CTXT = half[ic]
                    for qb in range(max(0, kb - 4 * ic), 4):
                        qh, offc = qb // 2, 128 * (qb % 2)
                        for ph in range(2):
                            h = 2 * pr + ph
                            lhsT = ATs[(pr, qh)][:, ph, offc : offc + 128]
                            nc.tensor.matmul(
                                CTXT[qb // 2][:, qb % 2,
                                              DH * h : DH * h + DH],
                                lhsT,
                                VA[:, kb, h, :],
                                start=False,
                                stop=False,
                                skip_group_check=True,
                            )
                            nc.tensor.matmul(
                                den_v[:, qb, h : h + 1],
                                lhsT,
                                ONES[:],
                                start=False,
                                stop=False,
                                skip_group_check=True,
                            )

                setup_half(0)
                prev = None
                for ic, kb in steps:
                    if ic == 1 and kb == 0:
                        setup_half(1)
                    c0 = 512 * ic
                    mhk, kbl = divmod(kb, 4)
                    j0 = max(c0, 128 * kb) - c0
                    diag = 128 * kb >= c0
                    qh_d = j0 // 256
                    ATs = {}
                    for pr in range(8):
                        if prev is not None:
                            emit_ctx_pr(prev[0], prev[1], prev[2], pr)
                        for qh in range(qh_d, 2):
                            off = max(j0 - 256 * qh, 0)
                            SW = pool.tile([128, 2, 256], F32, tag="sw",
                                           bufs=SW_BUFS, name="sw")
                            for ph in range(2):
                                h = 2 * pr + ph
                                nc.tensor.matmul(
                                    SW[:, ph, off:256],
                                    QKT8h[mhk][:, 1, :, h,
                                               128 * kbl : 128 * kbl + 128],
                                    QKT8h[ic][:, 0, :, h,
                                              256 * qh + off : 256 * (qh + 1)],
                                    start=True,
                                    stop=True,
                                    perf_mode=DR,
                                    skip_group_check=True,
                                )
                            AT = att.tile([128, 2, 256], BF16, tag="at",
                                          name="at")
                            xs.exp(
                                AT[:, :, off:256],
                                AT.bitcast(I16)[:, :, off:256],
                                SW[:, :, off:256],
                                2 * (256 - off),
                            )
                            if diag and qh == qh_d:
                                # late kbs: DVE 2x bf16 path; exp traffic
                                # is light there and Pool serializes
                                meng = (nc.vector if (ic == 1 and kb >= 6)
                                        else nc.gpsimd)
                                meng.tensor_tensor(
                                    AT[:, :, off : off + 128],
                                    AT[:, :, off : off + 128],
                                    _bcast(MASK, 1, 2),
                                    op=MULT,
                                )
                                xs.add("pool", 460)
                            ATs[(pr, qh)] = AT
                    if prev is not None and prev[1] - 4 * prev[0] >= 0:
                        normalize(prev[0], prev[1] - 4 * prev[0])
                    prev = (ic, kb, ATs)
                    fill_step()
                for pr in range(8):
                    emit_ctx_pr(prev[0], prev[1], prev[2], pr)
                normalize(prev[0], prev[1] - 4 * prev[0])
                while (fill_state["pending"] is not None
                       or fill_state["i"] < len(fill_items)):
                    fill_step()

            # ---- P3: transpose + output projection tail ----
            with (
                tc.tile_pool(name="ps_t", bufs=4, space="PSUM") as ps_t,
                tc.tile_pool(name="ps_o", bufs=2, space="PSUM") as ps_o,
            ):
                for qq in range(8):
                    CT = work.tile([128, 2, 128], BF16, tag="ct", name="ct")
                    for ch in range(2):
                        TP = ps_t.tile([128, 128], BF16, tag="tp", name="tp")
                        nc.tensor.matmul(
                            TP[:],
                            CN[:, qq, 128 * ch : 128 * ch + 128],
                            IDENT[:],
                            is_transpose=True,
                        )
                        nc.vector.tensor_copy(CT[:, ch, :], TP[:])
                    PO = ps_o.tile([128, EMB], F32, tag="po", name="po")
                    for nh in range(2):
                        for ch in range(2):
                            nc.tensor.matmul(
                                PO[:, 512 * nh : 512 * nh + 512],
                                CT[:, ch, :],
                                WO[:, ch, 512 * nh : 512 * nh + 512],
                                start=(ch == 0),
                                stop=(ch == 1),
                            )
                    Y = work.tile([128, EMB], BF16, tag="y", name="y")
                    for nh in range(2):
                        if (2 * qq + nh) % 2 == 0:
                            nc.scalar.copy(Y[:, 512 * nh : 512 * nh + 512],
                                           PO[:, 512 * nh : 512 * nh + 512])
                        else:
                            nc.vector.tensor_copy(
                                Y[:, 512 * nh : 512 * nh + 512],
                                PO[:, 512 * nh : 512 * nh + 512])
                    nc.sync.dma_start(y_d[128 * qq : 128 * qq + 128, :], Y[:])

    split_excess_waits(nc)
    return nc


_NC_CACHE = None


def _get_nc():
    global _NC_CACHE
    if _NC_CACHE is None:
        _NC_CACHE = build_nc()
    return _NC_CACHE


# column permutation: device col j = 16*e + h  <-  module-local col 16*h + e
_PERM = [(j % 16) * 16 + j // 16 for j in range(GCOLS)]


def _consts():
    idm = np.zeros((128, 256), dtype=BF)
    idm[:, 0:128] = np.eye(128, dtype=np.float32).astype(BF)
    idm[:, 128:256] = np.tril(
        np.ones((128, 128), dtype=np.float32)
    ).T.astype(BF)
    return idm


def kernel(x, Wq, Wk, Wv, Wo, bo):
    x = np.asarray(x, dtype=np.float32)
    Wq = np.asarray(Wq, dtype=np.float32)
    Wk = np.asarray(Wk, dtype=np.float32)
    Wv = np.asarray(Wv, dtype=np.float32)
    Wo = np.asarray(Wo, dtype=np.float32)
    bo = np.asarray(bo, dtype=np.float32)

    idm = _consts()
    nc = _get_nc()
    in_maps = []
    for c in range(8):
        b, g = divmod(c, NG)
        cols = slice(GCOLS * g, GCOLS * g + GCOLS)
        in_maps.append(
            {
                "xT": np.ascontiguousarray(x[b].T).astype(BF),
                "wq": np.ascontiguousarray(Wq[:, cols][:, _PERM]).astype(BF),
                "wk": np.ascontiguousarray(Wk[:, cols][:, _PERM]).astype(BF),
                "wv": np.ascontiguousarray(Wv[:, cols]).astype(BF),
                "wo": np.ascontiguousarray(Wo[cols, :]).astype(BF),
                "idm": idm,
            }
        )

    res = run_bass_kernel_spmd(nc, in_maps, core_ids=list(range(8)))
    out = np.zeros((BATCH, SEQ, EMB), dtype=np.float32)
    for c in range(8):
        b = c // NG
        out[b] += np.asarray(res.results[c]["y"], dtype=np.float32)
    out += bo[None, None, :]
    return out


# revision 54
# speedup vs baseline: 1.0017x; 1.0017x over previous
"""Multi-head attention (axis-swapped variant) on 8 Trainium2 NeuronCores.

Reference semantics (EMB=1024): 64 effective heads of size 16 acting on the
d_head axis, causal softmax scaled by 1/sqrt(16), projections Wq/Wk/Wv,
output projection Wo + bo.

Sharding: core c = 4*b + g handles batch b and head-group g (16 heads =
256 contiguous projection columns). Each core returns a partial output
[1024, 1024]; the host sums the 4 group partials per batch and adds bo.

Per-core pipeline:
- bf16 Q/K/V projections (fp32 PSUM accumulate); the first query-half Q/K
  wave runs first so the fp8 reshape roundtrip starts early
- Q/K evacuated to fp8e4m3 with columns pre-ordered (e,h); a DRAM
  roundtrip reshapes them to [8, qk, 2, 16, 512]-per-half so score
  matmuls run in fp8 DoubleRow mode (contraction 16 = 8 part x 2 k-tiles)
- causal diag masking post-exp on GPSIMD (SBUF-only engine) via a
  lower-triangular constant multiply
- softmax exp split across three engines (greedy balance): ACT true exp,
  DVE/Pool use the Schraudolph bit-trick (int16(x*A+B) bitcast to bf16);
  score tiles are single-PSUM-bank [128, 2, 256]
- ctx accumulated transposed (out [128 queries, 16] per head) plus
  free-size-1 denominator matmuls against a ones column
- one merged attention loop over both halves; second-half Q/K projection,
  V projection tail, and the per-query-block epilogue (PE transpose,
  bf16 out-projection, store) run as fillers through one spare PSUM bank
"""

import numpy as np
import ml_dtypes

import concourse.bass as bass
import concourse.mybir as mybir
import concourse.tile as tile
from concourse.bass_utils import run_bass_kernel_spmd

F32 = mybir.dt.float32
BF16 = mybir.dt.bfloat16
F8 = mybir.dt.float8e4
I16 = mybir.dt.int16
BF = ml_dtypes.bfloat16
F8NP = ml_dtypes.float8_e4m3

EMB = 1024
SEQ = 1024
BATCH = 2
NG = 4            # head groups (cores per batch)
HPG = 16          # heads per group/core
DH = 16           # per-head feature size
GCOLS = HPG * DH  # 256 projection columns per core

DR = mybir.MatmulPerfMode.DoubleRow
MULT = mybir.AluOpType.mult
ADD = mybir.AluOpType.add
EXPF = mybir.ActivationFunctionType.Exp

# Schraudolph exp: bf16(bitcast_int16(s * A_S + B_S)) ~= exp(0.25 * s)
A_S = float(np.float32(0.25 * 128.0 / np.log(2.0)))
B_S = 16248.0

N_WARM = 8  # PE p-state warmup matmuls while input DMAs land
SW_BUFS = 4
AT_BUFS = 40


def split_excess_waits(nc, cap=1):
    """This container's walrus rejects instructions carrying more than a few
    semaphore waits (and bass's own model says one). Relocate excess waits
    onto preceding same-engine EventSemaphore instructions."""

    def fix_block(bb, dummy):
        insts = bb.instructions
        i = 0
        while i < len(insts):
            inst = insts[i]
            si = inst.sync_info
            waits = list(si.on_wait) if si is not None and si.on_wait else []
            if len(waits) > cap:
                eng = nc.engines[inst.engine]
                excess, keep = waits[:-cap], waits[-cap:]
                si.on_wait = keep
                pos = i
                for j in range(0, len(excess), cap):
                    chunk = excess[j : j + cap]
                    ev = eng.wait_ge(dummy, 1)
                    cur_list = nc.cur_bb.bb.instructions
                    assert cur_list[-1] is ev.ins
                    cur_list.pop()
                    ev.ins.sync_info.on_wait = chunk
                    insts.insert(pos, ev.ins)
                    pos += 1
                    i += 1
            i += 1

    with nc.semaphore("waitfix_dummy") as dummy:
        for f in nc.m.functions:
            for bb in f.blocks:
                fix_block(bb, dummy)


def _bcast(ap, dim, count):
    """Insert a stride-0 dim at position `dim` of an AP."""
    new_ap = list(ap.ap)
    new_ap.insert(dim, [0, count])
    return bass.AP(tensor=ap.tensor, offset=ap.offset, ap=new_ap)


class VecSplit:
    """Greedy load-balancing of exp/copy work across ACT / DVE / Pool."""

    def __init__(self, nc):
        self.nc = nc
        self.load = {"act": 0.0, "dve": 0.0, "pool": 0.0}
        self.exp_i = 0
        self.exp_cost = {
            "act": lambda r: r * 0.8333 + 185.0,
            "dve": lambda r: r * 1.0417 + 125.0,
        }

    def exp(self, at, ati, sw, rows):
        eng = ("act", "dve")[self.exp_i % 2]
        self.exp_i += 1
        self.load[eng] += self.exp_cost[eng](rows)
        if eng == "act":
            self.nc.scalar.activation(at, sw, EXPF, scale=0.25)
        else:
            self.nc.vector.tensor_scalar(ati, sw, A_S, B_S, MULT, ADD)

    def copy(self, out, in_, costs):
        eng = min(costs, key=lambda e: self.load[e] + costs[e])
        self.load[eng] += costs[eng]
        if eng == "act":
            self.nc.scalar.copy(out, in_)
        elif eng == "dve":
            self.nc.vector.tensor_copy(out, in_)
        else:
            self.nc.gpsimd.tensor_copy(out, in_)
        return eng

    def add(self, eng, ns):
        self.load[eng] += ns


def build_nc():
    nc = bass.Bass()
    xT_d = nc.declare_dram_parameter("xT", [EMB, SEQ], BF16, isOutput=False)
    wq_d = nc.declare_dram_parameter("wq", [EMB, GCOLS], BF16, isOutput=False)
    wk_d = nc.declare_dram_parameter("wk", [EMB, GCOLS], BF16, isOutput=False)
    wv_d = nc.declare_dram_parameter("wv", [EMB, GCOLS], BF16, isOutput=False)
    wo_d = nc.declare_dram_parameter("wo", [GCOLS, EMB], BF16, isOutput=False)
    id_d = nc.declare_dram_parameter("idm", [128, 256], BF16, isOutput=False)
    y_d = nc.declare_dram_parameter("y", [SEQ, EMB], BF16, isOutput=True)

    with tile.TileContext(nc) as tc:
        with (
            tc.tile_pool(name="big", bufs=1) as big,
            tc.tile_pool(name="att", bufs=AT_BUFS) as att,
            tc.tile_pool(name="work", bufs=4) as work,
            tc.tile_pool(name="dram", bufs=1, space="DRAM") as dram,
        ):
            # ---- input DMAs (order = SP queue order; no waits on any) ----
            xT_r = xT_d[:].rearrange("(kb p) m -> p kb m", p=128)
            XT = big.tile([128, 8, SEQ], BF16)
            WQ = big.tile([128, 8, GCOLS], BF16)
            WK = big.tile([128, 8, GCOLS], BF16)
            WV = big.tile([128, 8, GCOLS], BF16)
            WO = big.tile([128, 2, EMB], BF16)
            IDM = big.tile([128, 256], BF16)

            def xchunk(ci, mh):
                sl = (slice(None), slice(2 * ci, 2 * ci + 2),
                      slice(512 * mh, 512 * mh + 512))
                nc.sync.dma_start(XT[sl], xT_r[sl])

            nc.sync.dma_start(WQ[:], wq_d[:].rearrange("(kb p) n -> p kb n", p=128))
            xchunk(0, 0)
            nc.sync.dma_start(WK[:], wk_d[:].rearrange("(kb p) n -> p kb n", p=128))
            xchunk(1, 0)
            xchunk(2, 0)
            xchunk(3, 0)
            nc.sync.dma_start(WV[:], wv_d[:].rearrange("(kb p) n -> p kb n", p=128))

            IDENT = IDM[:, 0:128]
            MASK = IDM[:, 128:256]   # lower-tri ones: key p <= query m

            QK8 = big.tile([128, 2, 2, SEQ], F8)       # (p=col, ct, qk, m)
            # per m-half fp8 score operands: (p8, qk, i, h, m)
            QKT8h = [
                big.tile([8, 2, 2, HPG, 512], F8, name=f"qkt8h{mh}")
                for mh in range(2)
            ]
            VA = big.tile([128, 8, HPG, DH], BF16)     # (p=key, kb, h, e)
            ONES = big.tile([128, 1], BF16)
            nc.gpsimd.memset(ONES[:], 1.0)
            ZL = big.tile([8, 2, 128], F8)
            nc.gpsimd.memset(ZL[:], 0.0)
            ZR = big.tile([8, 2, 512], F8)
            nc.gpsimd.memset(ZR[:], 0.0)
            CN = big.tile([128, 8, GCOLS], BF16)       # normalized ctx per qq
            qk8_d = dram.tile([2, 2, 2, 128, 512], F8)  # (mh, qk, ct, p, m)

            xs = VecSplit(nc)

            def zero_mm(out_ap):
                nc.tensor.matmul(out_ap, ZL[:], ZR[:], start=True, stop=False,
                                 perf_mode=DR, skip_group_check=True)

            def rt_dma(qki, mh):
                eng = nc.scalar if (qki == 0 and mh == 0) else nc.sync
                eng.dma_start(
                    qk8_d[mh, qki].rearrange("ct p m -> p ct m"),
                    QK8[:, :, qki, 512 * mh : 512 * mh + 512],
                )
                eng.dma_start(
                    QKT8h[mh][:, qki],
                    qk8_d[mh, qki].rearrange("i (p8 h) m -> p8 i h m", p8=8),
                )

            def proj_mms(pq, qki, ct, mh):
                Wt = (WQ, WK)[qki]
                for kb in range(8):
                    nc.tensor.matmul(
                        pq[:, 0:512],
                        Wt[:, kb, 128 * ct : 128 * ct + 128],
                        XT[:, kb, 512 * mh : 512 * mh + 512],
                        start=(kb == 0),
                        stop=(kb == 7),
                    )

            def proj_evac(pq, qki, ct, mh):
                xs.copy(
                    QK8[:, ct, qki, 512 * mh : 512 * mh + 512], pq[:, 0:512],
                    {"act": 612.0, "dve": 660.0},
                )
                if ct == 1:
                    rt_dma(qki, mh)

            def v_mms(pv, mt):
                for kb in range(8):
                    nc.tensor.matmul(
                        pv[:, 0:GCOLS],
                        XT[:, kb, 128 * mt : 128 * mt + 128],
                        WV[:, kb, :],
                        start=(kb == 0),
                        stop=(kb == 7),
                    )

            def v_evac(pv, mt):
                xs.copy(
                    VA[:, mt, :, :],
                    pv[:, 0:GCOLS].rearrange("p (h e) -> p h e", e=DH),
                    {"dve": 520.0, "act": 560.0},
                )

            # ---- P0: PE p-state warmup on zeros while DMAs land ----
            with tc.tile_pool(name="ps_w", bufs=1, space="PSUM") as ps_w:
                WARM = ps_w.tile([128, 512], F32, tag="warm")
                for _ in range(N_WARM):
                    nc.tensor.matmul(WARM[:], ZL[:], ZR[:], start=True, stop=True,
                                     perf_mode=DR, skip_group_check=True)

            # ---- P1: Q/K proj mh0 wave, then V mt0..3 ----
            with tc.tile_pool(name="ps_p", bufs=1, space="PSUM") as ps_p:
                pqs = {}
                for qki in range(2):
                    for ct in range(2):
                        pqs[(qki, ct)] = ps_p.tile(
                            [128, 512], F32, tag=f"pp{qki}{ct}",
                            name=f"pq{qki}{ct}",
                        )
                for kb in range(8):
                    for qki, Wt in enumerate((WQ, WK)):
                        for ct in range(2):
                            nc.tensor.matmul(
                                pqs[(qki, ct)][:],
                                Wt[:, kb, 128 * ct : 128 * ct + 128],
                                XT[:, kb, 0:512],
                                start=(kb == 0),
                                stop=(kb == 7),
                            )
                for qki in range(2):
                    for ct in range(2):
                        xs.copy(
                            QK8[:, ct, qki, 0:512], pqs[(qki, ct)][:],
                            {("act", "dve")[qki]: (612.0, 660.0)[qki]},
                        )
                    rt_dma(qki, 0)
                for ci in range(4):
                    xchunk(ci, 1)
                nc.sync.dma_start(IDM[:], id_d[:])
                nc.sync.dma_start(
                    WO[:], wo_d[:].rearrange("(ch p) n -> p ch n", p=128))
                for mt in range(4):
                    pv = ps_p.tile([128, 512], F32, tag="pv", name=f"pv{mt}",
                                   bufs=2)
                    v_mms(pv, mt)
                    v_evac(pv, mt)

            # ---- P2: merged attention over both halves + fillers ----
            with tc.tile_pool(name="ps_a", bufs=1, space="PSUM") as pool:

                fill_items = [("p", 0, 0), ("p", 0, 1), ("p", 1, 0),
                              ("p", 1, 1), ("v", 4, 0), ("v", 5, 0),
                              ("v", 6, 0), ("v", 7, 0)]
                fill_state = {"i": 0, "pending": None}

                def fill_step():
                    if fill_state["pending"] is not None:
                        kind, a, b, tile_ = fill_state["pending"]
                        v_evac(tile_, a)
                        fill_state["pending"] = None
                    i = fill_state["i"]
                    if i < len(fill_items):
                        fill_state["i"] = i + 1
                        if fill_items[i] is None:
                            return
                        kind, a, b = fill_items[i]
                        t = pool.tile([128, 512], F32, tag="fill", bufs=1,
                                      name="fillt")
                        if kind == "p":
                            # early iterations, exp traffic still light:
                            # evac+rt immediately so QKT8h[1] lands in time
                            proj_mms(t, a, b, 1)
                            proj_evac(t, a, b, 1)
                        else:
                            v_mms(t, a)
                            fill_state["pending"] = (kind, a, b, t)

                steps = [(0, kb) for kb in range(4)] + \
                        [(1, kb) for kb in range(8)]

                half = {}

                def setup_half(ic):
                    DEN = pool.tile([128, 512], F32, tag="den", bufs=1,
                                    name=f"den{ic}")
                    zero_mm(DEN[:])
                    CTXT = []
                    for hf in range(2):
                        t = pool.tile([128, 2, GCOLS], F32, tag="ctx", bufs=2,
                                      name=f"ctx{ic}{hf}")
                        zero_mm(t[:])
                        CTXT.append(t)
                    half[ic] = (
                        DEN[:, 0:64].rearrange("p (qb h) -> p qb h", h=HPG),
                        CTXT,
                    )

                def normalize(ic, qb):
                    den_v, CTXT = half[ic]
                    qq = 4 * ic + qb
                    ctx3 = CTXT[qb // 2][:, qb % 2, :].rearrange(
                        "p (h e) -> p h e", e=DH)
                    RG = work.tile([128, HPG], F32, tag="rg", name="rg")
                    nc.vector.reciprocal(out=RG[:], in_=den_v[:, qb, :])
                    nc.vector.tensor_tensor(
                        CN[:, qq, :].rearrange("p (h e) -> p h e", e=DH),
                        ctx3,
                        _bcast(RG[:], 2, DH),
                        op=MULT,
                    )
                    xs.add("dve", 900)

                def emit_ctx_pr(ic, kb, ATs, pr):
                    den_v, CTXT = half[ic]
                    for qb in range(max(0, kb - 4 * ic), 4):
                        qh, offc = qb // 2, 128 * (qb % 2)
                        for ph in range(2):
                            h = 2 * pr + ph
                            lhsT = ATs[(pr, qh)][:, ph, offc : offc + 128]
                            nc.tensor.matmul(
                                CTXT[qb // 2][:, qb % 2,
                                              DH * h : DH * h + DH],
                                lhsT,
                                VA[:, kb, h, :],
                                start=False,
                                stop=False,
                                skip_group_check=True,
                            )
                            nc.tensor.matmul(
                                den_v[:, qb, h : h + 1],
                                lhsT,
                                ONES[:],
                                start=False,
                                stop=False,
                                skip_group_check=True,
                            )

                setup_half(0)
                prev = None
                for ic, kb in steps:
                    if ic == 1 and kb == 0:
                        setup_half(1)
                    c0 = 512 * ic
                    mhk, kbl = divmod(kb, 4)
                    j0 = max(c0, 128 * kb) - c0
                    diag = 128 * kb >= c0
                    qh_d = j0 // 256
                    ATs = {}
                    for pr in range(8):
                        if prev is not None:
                            emit_ctx_pr(prev[0], prev[1], prev[2], pr)
                        for qh in range(qh_d, 2):
                            off = max(j0 - 256 * qh, 0)
                            SW = pool.tile([128, 2, 256], F32, tag="sw",
                                           bufs=SW_BUFS, name="sw")
                            for ph in range(2):
                                h = 2 * pr + ph
                                nc.tensor.matmul(
                                    SW[:, ph, off:256],
                                    QKT8h[mhk][:, 1, :, h,
                                               128 * kbl : 128 * kbl + 128],
                                    QKT8h[ic][:, 0, :, h,
                                              256 * qh + off : 256 * (qh + 1)],
                                    start=True,
                                    stop=True,
                                    perf_mode=DR,
                                    skip_group_check=True,
                                )
                            AT = att.tile([128, 2, 256], BF16, tag="at",
                                          name="at")
                            xs.exp(
                                AT[:, :, off:256],
                                AT.bitcast(I16)[:, :, off:256],
                                SW[:, :, off:256],
                                2 * (256 - off),
                            )
                            if diag and qh == qh_d:
                                # late kbs: DVE 2x bf16 path; exp traffic
                                # is light there and Pool serializes
                                meng = (nc.vector if (ic == 1 and kb >= 6)
                                        else nc.gpsimd)
                                meng.tensor_tensor(
                                    AT[:, :, off : off + 128],
                                    AT[:, :, off : off + 128],
                                    _bcast(MASK, 1, 2),
                                    op=MULT,
                                )
                                xs.add("pool", 460)
                            ATs[(pr, qh)] = AT
                    if prev is not None and prev[1] - 4 * prev[0] >= 0:
                        normalize(prev[0], prev[1] - 4 * prev[0])
                    prev = (ic, kb, ATs)
                    fill_step()
                for pr in range(8):
                    emit_ctx_pr(prev[0], prev[1], prev[2], pr)
                normalize(prev[0], prev[1] - 4 * prev[0])
                while (fill_state["pending"] is not None
                       or fill_state["i"] < len(fill_items)):
                    fill_step()

            # ---- P3: transpose + output projection tail ----
            with (
                tc.tile_pool(name="ps_t", bufs=4, space="PSUM") as ps_t,
                tc.tile_pool(name="ps_o", bufs=2, space="PSUM") as ps_o,
            ):
                for qq in range(8):
                    CT = work.tile([128, 2, 128], BF16, tag="ct", name="ct")
                    for ch in range(2):
                        TP = ps_t.tile([128, 128], BF16, tag="tp", name="tp")
                        nc.tensor.matmul(
                            TP[:],
                            CN[:, qq, 128 * ch : 128 * ch + 128],
                            IDENT[:],
                            is_transpose=True,
                        )
                        nc.vector.tensor_copy(CT[:, ch, :], TP[:])
                    PO = ps_o.tile([128, EMB], F32, tag="po", name="po")
                    for nh in range(2):
                        for ch in range(2):
                            nc.tensor.matmul(
                                PO[:, 512 * nh : 512 * nh + 512],
                                CT[:, ch, :],
                                WO[:, ch, 512 * nh : 512 * nh + 512],
                                start=(ch == 0),
                                stop=(ch == 1),
                            )
                    Y = work.tile([128, EMB], BF16, tag="y", name="y")
                    for nh in range(2):
                        if (2 * qq + nh) % 2 == 0:
                            nc.scalar.copy(Y[:, 512 * nh : 512 * nh + 512],
                                           PO[:, 512 * nh : 512 * nh + 512])
                        else:
                            nc.vector.tensor_copy(
                                Y[:, 512 * nh : 512 * nh + 512],
                                PO[:, 512 * nh : 512 * nh + 512])
                    nc.sync.dma_start(y_d[128 * qq : 128 * qq + 128, :], Y[:])

    split_excess_waits(nc)
    return nc


_NC_CACHE = None


def _get_nc():
    global _NC_CACHE
    if _NC_CACHE is None:
        _NC_CACHE = build_nc()
    return _NC_CACHE


# column permutation: device col j = 16*e + h  <-  module-local col 16*h + e
_PERM = [(j % 16) * 16 + j // 16 for j in range(GCOLS)]


def _consts():
    idm = np.zeros((128, 256), dtype=BF)
    idm[:, 0:128] = np.eye(128, dtype=np.float32).astype(BF)
    idm[:, 128:256] = np.tril(
        np.ones((128, 128), dtype=np.float32)
    ).T.astype(BF)
    return idm


def kernel(x, Wq, Wk, Wv, Wo, bo):
    x = np.asarray(x, dtype=np.float32)
    Wq = np.asarray(Wq, dtype=np.float32)
    Wk = np.asarray(Wk, dtype=np.float32)
    Wv = np.asarray(Wv, dtype=np.float32)
    Wo = np.asarray(Wo, dtype=np.float32)
    bo = np.asarray(bo, dtype=np.float32)

    idm = _consts()
    nc = _get_nc()
    in_maps = []
    for c in range(8):
        b, g = divmod(c, NG)
        cols = slice(GCOLS * g, GCOLS * g + GCOLS)
        in_maps.append(
            {
                "xT": np.ascontiguousarray(x[b].T).astype(BF),
                "wq": np.ascontiguousarray(Wq[:, cols][:, _PERM]).astype(BF),
                "wk": np.ascontiguousarray(Wk[:, cols][:, _PERM]).astype(BF),
                "wv": np.ascontiguousarray(Wv[:, cols]).astype(BF),
                "wo": np.ascontiguousarray(Wo[cols, :]).astype(BF),
                "idm": idm,
            }
        )

    res = run_bass_kernel_spmd(nc, in_maps, core_ids=list(range(8)))
    out = np.zeros((BATCH, SEQ, EMB), dtype=np.float32)
    for c in range(8):
        b = c // NG
        out[b] += np.asarray(res.results[c]["y"], dtype=np.float32)
    out += bo[None, None, :]
    return out
